# revision 1
# baseline (speedup 1.0000x reference)
"""BasicSSM Trainium2 kernel (bf16 datapath + XBAR DMA transpose).

Math: A_bar = expm(delta*A); u = x @ (delta*B)^T; h_t = h_{t-1} @ A_bar^T + u_t;
y = h @ C^T.

Because A = 0.05*randn - 0.5*I (documented construction in the reference), the
spectral radius of P = A_bar^T is ~0.65-0.75, so P^d decays below the bf16
noise floor by d ~ 32.  The scan is computed as a windowed convolution
    H[s] = sum_{d=0}^{W-1} u[s-d] @ P^d          (W = 8*N_D0 lags, adaptive)
which makes sequence sharding communication-free (each core only needs a
W-row halo of x).

Sharding: 8 cores = 4 batches x 2 sequence halves (communication-free).
The whole device datapath is bf16 (x and y are transported as bf16; PSUM
accumulation stays fp32), which halves DMA traffic vs fp32 — measured
end-to-end error ~3e-3 norm-relative, well inside the 2e-2 gate.

Per core (x slice is HP=64 halo rows + 2048 rows, zero-padded at t=0):
  stage 1: per 512-row span, 8 XBAR DMA-transposes (16x128-tile crossbar
           path, bf16-only) load x^T d-chunks straight from HBM into SBUF
           (no PE transposes, no PSUM->SBUF copies) -> 8 accumulating
           matmuls -> u^T master (16, 2112)
  stage 2: per 512-col window, ONE overlapping-AP SBUF->SBUF DMA builds an
           8-lag-stacked tile u8[(m,d_rev), j] = u^T[m, base+d_rev+j]; N_D0
           accumulating matmuls against host-built P-power stacks -> H^T
  stage 3: y tile (128,1024) = H^T_slice.T @ C^T (PSUM) -> bf16 copy -> HBM
Scheduling: the timing loop body is software-pipelined — iteration k's
stage-3 (reading the loop-carried htm of k-1, pool bufs=2) interleaves
into iteration k's stage-1 spans, and x^T spans live in a 10-deep tile
pool so the XBAR load train prefetches ~2 iterations ahead and the DMA
pool never starves.  Queue roles: SP carries the readiness-monotonic
load train (xbars, then u8 builds), ACT carries stores (+half the PSUM
copies), DVE the rest; per-queue issue order matches readiness order to
avoid FIFO head-of-line blocking, and few, large DMAs keep the 8 shared
hardware DGE rings off the critical path.  PSUM: 1 bank u, 1 bank h,
6-deep psy pool.  Consts ride SWDGE so the first xbar wins the DMA pool
immediately.
"""

import numpy as np
import ml_dtypes

BF16 = np.dtype(ml_dtypes.bfloat16)

D_MODEL = 1024
D_STATE = 16
BATCH = 4
SEQ = 4096
N_CORES = 8
HALF = SEQ // 2           # 2048 rows of output per core
HP = 64                   # halo rows (supports window up to 7*8 = 56 lags)
ROWS = HP + HALF          # 2112
NYT = HALF // 128         # 16 y-tiles
NW = HALF // 512          # 4 scan windows of 512
N_D0 = 4                  # 8-lag groups -> window W = 32 lags (adaptive,
                          # widened at run time if P decays slowly; HP=64
                          # supports N_D0 <= 7)
U8F = 512 + 8 * N_D0 - 1  # u8 tile free size
LM = 8 * N_D0 - 1         # left margin inside u8 tile
SPANS = [(0, 512), (512, 512), (1024, 512), (1536, 512), (2048, HP)]


def _set_window(n_d0):
    global N_D0, U8F, LM
    N_D0 = n_d0
    U8F = 512 + 8 * N_D0 - 1
    LM = 8 * N_D0 - 1

_CACHE = {}
LAST_RESULTS = None  # BassKernelResults from the most recent run (for profiling)
TRACE = False


def _expm(M):
    """Scaling-and-squaring Taylor expm in float64 (16x16, ||M|| ~ 0.7)."""
    M = np.asarray(M, dtype=np.float64)
    nrm = np.linalg.norm(M, 1)
    s = max(0, int(np.ceil(np.log2(max(nrm, 1e-300)))) + 1) if nrm > 0.5 else 0
    Ms = M / (2.0 ** s)
    E = np.eye(M.shape[0])
    T = np.eye(M.shape[0])
    for k in range(1, 40):
        T = T @ Ms / k
        E = E + T
    for _ in range(s):
        E = E @ E
    return E


def _build_program(loop_n=None):
    """Build the (shared, SPMD) Bass program.  loop_n=None: one-shot
    correctness program (external xs/ys).  loop_n=int: hardware-loop timing
    variant (For_i, body = 2 software-pipelined logical iterations, xs/ys
    internal so dispatch cost is negligible).  loop_n="unrollN": straight-
    line N-iteration variant used by the timeline-sim harness."""

    import concourse.bass as bass
    import concourse.bacc as bacc
    import concourse.mybir as mybir
    import concourse.tile as tile

    f32 = mybir.dt.float32
    bf16 = mybir.dt.bfloat16
    nc = bacc.Bacc(
        "TRN2", target_bir_lowering=False, debug=False, num_devices=N_CORES
    )

    if loop_n is None:
        xs = nc.dram_tensor("xs", [ROWS, D_MODEL], bf16, kind="ExternalInput")
        ys = nc.dram_tensor("ys", [HALF, D_MODEL], bf16, kind="ExternalOutput")
    else:
        xs = nc.dram_tensor("xs", [ROWS, D_MODEL], bf16)
        ys = nc.dram_tensor("ys", [HALF, D_MODEL], bf16)
        done = nc.dram_tensor("done", [128, 1], bf16, kind="ExternalOutput")
    bbt = nc.dram_tensor("bbt", [D_MODEL, D_STATE], bf16, kind="ExternalInput")
    pc = nc.dram_tensor("pc", [128, N_D0 * D_STATE], bf16, kind="ExternalInput")
    ct = nc.dram_tensor("ct", [D_STATE, D_MODEL], bf16, kind="ExternalInput")

    with tile.TileContext(nc) as tc:
        with (
            tc.tile_pool(name="consts", bufs=1) as consts,
            tc.tile_pool(name="masters", bufs=2) as masters,
            tc.tile_pool(name="xtsp", bufs=10) as xtsp,
            tc.tile_pool(name="u8", bufs=2) as u8p,
            tc.tile_pool(name="yout", bufs=4) as youtp,
            tc.tile_pool(name="ps_u", bufs=1, space=bass.MemorySpace.PSUM) as ps_u,
            tc.tile_pool(name="ps_h", bufs=1, space=bass.MemorySpace.PSUM) as ps_h,
            tc.tile_pool(name="ps_y", bufs=6, space=bass.MemorySpace.PSUM) as ps_y,
        ):
            # --- constants ---
            bbt_s = consts.tile([128, 8, D_STATE], bf16)  # (dpart, kchunk, n)
            nc.sync.dma_start(
                bbt_s[:], bbt[:].rearrange("(k p) n -> p k n", p=128)
            )
            pc_s = consts.tile([128, N_D0 * D_STATE], bf16)
            nc.gpsimd.dma_start(pc_s[:], pc[:])
            ct_s = consts.tile([D_STATE, D_MODEL], bf16)
            nc.gpsimd.dma_start(ct_s[:], ct[:])
            # warm the ACT activation-function table off the critical path
            warm = consts.tile([1, 2], bf16)
            nc.scalar.copy(warm[:, 1:2], warm[:, 0:1])

            state = {}

            # stage-1 load: ONE XBAR DMA-transpose per span moves
            # x[r0:r0+rn, :] -> xtm[p, i, c, j] = x[r0+j, c*128+p]
            # (16x128-tile crossbar path, bf16-only)
            def xbar(i):
                r0, rn = SPANS[i]
                nc.sync.dma_start_transpose(
                    state["xtm"][i][:, :, :rn], xs[r0:r0 + rn, :]
                )

            # stage-1 compute: u^T[:, r0:r0+rn] = Bb @ x[r0:r0+rn, :]^T
            def st(i):
                r0, rn = SPANS[i]
                psu = ps_u.tile([D_STATE, 512], f32, tag="psu")
                for cc in range(8):
                    nc.tensor.matmul(
                        psu[:, :rn],
                        bbt_s[:, cc, :],
                        state["xtm"][i][:, cc, :rn],
                        start=(cc == 0),
                        stop=(cc == 7),
                    )
                nc.vector.tensor_copy(state["utm"][:, r0:r0 + rn], psu[:, :rn])

            # stage-2 window pair: H^T[:, 1024v:1024v+1024]; ONE u8 build
            # (overlapping-AP DMA stacks 8 lags into partitions; d reversed
            # so the shift step is +1; reversal baked into pc on the host)
            def win2(v):
                w0 = HP + 1024 * v
                u8f2 = LM + 1024
                utm = state["utm"]
                u8 = u8p.tile([128, u8f2], bf16, tag="u8")
                utm_base = utm[:, 0:1]
                src = bass.AP(
                    utm_base.tensor,
                    utm_base.offset + (w0 - LM - 7),
                    [[ROWS, D_STATE], [1, 8], [1, u8f2]],
                )
                nc.sync.dma_start(u8[:], src)
                for h in range(2):
                    psh = ps_h.tile([D_STATE, 512], f32, tag="psh")
                    for d0 in range(N_D0):
                        off = LM - 8 * d0 + 512 * h
                        nc.tensor.matmul(
                            psh[:],
                            pc_s[:, d0 * D_STATE:(d0 + 1) * D_STATE],
                            u8[:, off:off + 512],
                            start=(d0 == 0),
                            stop=(d0 == N_D0 - 1),
                        )
                    w = 2 * v + h
                    if h == 0:
                        nc.vector.tensor_copy(
                            state["htm"][:, w * 512:(w + 1) * 512], psh[:]
                        )
                    else:
                        nc.scalar.copy(
                            state["htm"][:, w * 512:(w + 1) * 512], psh[:]
                        )

            # stage-3 quarter: 4 y-tiles from src_htm into ybufs[G//2];
            # after the second quarter of a half, ONE 1024-row store DMA
            # (row tt*128+p <- ybuf[p, tt, :])
            def y4(G, src_htm, ybufs):
                H, q = divmod(G, 2)
                ybuf = ybufs[H]
                for t4 in range(4):
                    tt = 4 * q + t4
                    t = 8 * H + tt
                    for g in range(2):
                        psy = ps_y.tile([128, 512], f32, tag="psy")
                        nc.tensor.matmul(
                            psy[:],
                            src_htm[:, t * 128:(t + 1) * 128],
                            ct_s[:, g * 512:(g + 1) * 512],
                            start=True,
                            stop=True,
                        )
                        dst = ybuf[:, tt, g * 512:(g + 1) * 512]
                        if g == 0:
                            nc.vector.tensor_copy(dst, psy[:])
                        else:
                            nc.scalar.copy(dst, psy[:])
                if q == 1:
                    nc.scalar.dma_start(
                        ys[H * 1024:(H + 1) * 1024, :].rearrange(
                            "(tt p) c -> p tt c", p=128
                        ),
                        ybuf[:],
                    )

            def alloc_ybufs():
                yb0 = youtp.tile([128, 8, D_MODEL], bf16, tag="ybuf", name="yb0")
                yb1 = youtp.tile([128, 8, D_MODEL], bf16, tag="ybuf", name="yb1")
                return [yb0, yb1]

            # one logical iteration; y-phase of prev_htm is interleaved
            # into this iteration's pool-paced stage-1 spans
            def schedule(prev_htm, flush):
                utm_t = masters.tile([D_STATE, ROWS], bf16, tag="utm", name="utm_t")
                state["utm"] = utm_t
                htm_t = masters.tile([D_STATE, HALF], bf16, tag="htm", name="htm_t")
                state["htm"] = htm_t
                spans_t = []
                for i in range(5):
                    sp_t = xtsp.tile([128, 8, 512], bf16, tag="xtsp",
                                     name=f"sp_t")
                    spans_t.append(sp_t)
                state["xtm"] = spans_t
                for i in range(5):
                    xbar(i)
                if prev_htm is not None:
                    ybufs = alloc_ybufs()
                    st(0)
                    y4(0, prev_htm, ybufs)
                    st(1)
                    y4(1, prev_htm, ybufs)
                    st(2)
                    win2(0)
                    y4(2, prev_htm, ybufs)
                    st(3)
                    y4(3, prev_htm, ybufs)
                    st(4)
                    win2(1)
                else:
                    for i in range(3):
                        st(i)
                    win2(0)
                    st(3)
                    st(4)
                    win2(1)
                cur = state["htm"]
                if flush:
                    ybufs = alloc_ybufs()
                    for G in range(4):
                        y4(G, cur, ybufs)
                return cur

            if loop_n is None:
                schedule(None, flush=True)
            elif isinstance(loop_n, str) and loop_n.startswith("unroll"):
                n = int(loop_n[6:])
                prev = None
                for k in range(n):
                    prev = schedule(prev, flush=(k == n - 1))
                nc.sync.dma_start(done[:], pc_s[:, 0:1])
            else:
                # hardware-loop timing variant: body = 2 logical iterations
                # (pool rotation consistent across the loop boundary);
                # htm pre-allocated so the body's first y-phase has a
                # loop-carried source (garbage data on the first pass —
                # timing only, ys is never read)
                prev = masters.tile([D_STATE, HALF], bf16, tag="htm")
                nc.gpsimd.memset(prev[:], 0)
                with tc.For_i(0, loop_n, 1):
                    prev = schedule(prev, flush=False)
                    prev = schedule(prev, flush=False)
                nc.sync.dma_start(done[:], pc_s[:, 0:1])

    nc.compile()
    return nc


def _get_runner(nc):
    """Cached shard_map runner (mirrors bass2jax.run_bass_via_pjrt but the
    jitted callable persists across kernel() calls)."""
    import jax
    import numpy as _np
    from jax.sharding import Mesh, PartitionSpec
    try:
        from jax.experimental.shard_map import shard_map
    except ImportError:
        from jax.shard_map import shard_map
    import concourse.mybir as mybir
    from concourse import bass2jax

    bass2jax.install_neuronx_cc_hook()
    part_name = nc.partition_id_tensor.name if nc.partition_id_tensor else None
    in_names, out_names, out_avals, zero_outs = [], [], [], []
    for alloc in nc.m.functions[0].allocations:
        if not isinstance(alloc, mybir.MemoryLocationSet):
            continue
        name = alloc.memorylocations[0].name
        if alloc.kind == "ExternalInput":
            if name != part_name:
                in_names.append(name)
        elif alloc.kind == "ExternalOutput":
            shape = tuple(alloc.tensor_shape)
            dtype = mybir.dt.np(alloc.dtype)
            out_names.append(name)
            out_avals.append(jax.core.ShapedArray(shape, dtype))
            zero_outs.append(_np.zeros(shape, dtype))
    n_params = len(in_names)
    n_outs = len(out_avals)
    all_names = in_names + out_names
    if part_name is not None:
        all_names = all_names + [part_name]
    donate = tuple(range(n_params, n_params + n_outs))

    def _body(*args):
        operands = list(args)
        if part_name is not None:
            operands.append(bass2jax.partition_id_tensor())
        outs = bass2jax._bass_exec_p.bind(
            *operands,
            out_avals=tuple(out_avals),
            in_names=tuple(all_names),
            out_names=tuple(out_names),
            lowering_input_output_aliases=(),
            sim_require_finite=True,
            sim_require_nnan=True,
            nc=nc,
        )
        return tuple(outs)

    devices = jax.devices()[:N_CORES]
    mesh = Mesh(np.asarray(devices), ("core",))
    specs = (PartitionSpec("core"),) * (n_params + n_outs)
    sharded = jax.jit(
        shard_map(_body, mesh=mesh, in_specs=specs,
                  out_specs=(PartitionSpec("core"),) * n_outs, check_rep=False),
        donate_argnums=donate, keep_unused=True,
    )
    return sharded, in_names, out_names, zero_outs


def _run_spmd_cached(nc, in_maps):
    import jax
    if "runner" not in _CACHE:
        _CACHE["runner"] = _get_runner(nc)
    sharded, in_names, out_names, zero_outs = _CACHE["runner"]
    concat_in = [
        np.concatenate([np.asarray(in_maps[c][n]) for c in range(N_CORES)], axis=0)
        for n in in_names
    ]
    concat_zero = [np.concatenate([z] * N_CORES, axis=0) for z in zero_outs]
    outs = sharded(*concat_in, *concat_zero)
    outs = [np.asarray(o) for o in outs]
    results = []
    for c in range(N_CORES):
        m = {}
        for i, n in enumerate(out_names):
            per = outs[i].shape[0] // N_CORES
            m[n] = outs[i][c * per:(c + 1) * per]
        results.append(m)
    return results


def bench_hw(x, A, B, C, delta, n=2048, n0=1024):
    """Absolute HW timing via a For_i-looped variant of the program with
    internal xs/ys (tiny external I/O).  Returns (times, per_iter_seconds)."""
    import time as _time
    import jax
    kernel(x, A, B, C, delta)  # fills _CACHE["last_in_maps"]
    in_maps = _CACHE["last_in_maps"]

    results = {}
    for n_iter in (n0, n):
        key = f"loopnc_{n_iter}"
        if key not in _CACHE:
            _CACHE[key] = _build_program(loop_n=n_iter)
            _CACHE[key + "_runner"] = _get_runner(_CACHE[key])
        ncl = _CACHE[key]
        sharded, in_names, out_names, zero_outs = _CACHE[key + "_runner"]
        concat_in = [
            np.concatenate(
                [np.asarray(in_maps[c][nm]) for c in range(N_CORES)], axis=0
            )
            for nm in in_names
        ]
        best = 1e9
        for rep in range(14):
            concat_zero = [np.concatenate([z] * N_CORES, axis=0) for z in zero_outs]
            t0 = _time.time()
            r = sharded(*concat_in, *concat_zero)
            jax.block_until_ready(r)
            dt = _time.time() - t0
            if rep > 0:
                best = min(best, dt)
        results[n_iter] = best
    per_iter = (results[n] - results[n0]) / (n - n0) / 2.0  # body = 2 iters
    return results, per_iter


def kernel(x, A, B, C, delta):
    global LAST_RESULTS
    from concourse.bass_utils import run_bass_kernel_spmd

    x = np.ascontiguousarray(np.asarray(x, dtype=np.float32))
    dl = float(np.asarray(delta).reshape(-1)[0])

    # host-side tiny-weight prep (float64)
    A_bar = _expm(dl * np.asarray(A, np.float64))       # (N, N)
    P = A_bar.T
    pows = [np.eye(D_STATE)]
    for _ in range(8 * 7):
        pows.append(pows[-1] @ P)
    # widen the window if P^(8*N_D0) hasn't decayed below bf16 significance
    want = 4
    while want < 7 and np.linalg.norm(pows[8 * want], 2) > 2e-4:
        want += 1
    if want != N_D0:
        _set_window(want)
        _CACHE.clear()
    # u8 partition layout is (m, d_rev) = m*8 + d_rev (partition-major DMA
    # legality) with d reversed so the shift step is +1; pc rows match:
    # pc[m*8 + d_rev, d0*16 + n] = P^(8*d0 + 7 - d_rev)[m, n]
    pc_np = np.zeros((128, N_D0 * D_STATE), np.float32)
    for d0 in range(N_D0):
        for dr in range(8):
            for m in range(D_STATE):
                pc_np[m * 8 + dr, d0 * D_STATE:(d0 + 1) * D_STATE] = \
                    pows[8 * d0 + 7 - dr][m].astype(np.float32)
    pc_np = pc_np.astype(BF16)
    bbt_np = np.ascontiguousarray(
        (dl * np.asarray(B, np.float64)).T.astype(np.float32)
    ).astype(BF16)
    ct_np = np.ascontiguousarray(np.asarray(C, np.float32).T).astype(BF16)

    if "nc" not in _CACHE:
        _CACHE["nc"] = _build_program()
    nc = _CACHE["nc"]
    assert np.linalg.norm(pows[8 * N_D0], 2) <= 5e-3, "window too short for this A"

    xbf = x.astype(BF16)
    in_maps = []
    for core in range(N_CORES):
        b, half = divmod(core, 2)
        t0 = half * HALF
        xs_np = np.zeros((ROWS, D_MODEL), BF16)
        if t0 >= HP:
            xs_np[:HP] = xbf[b, t0 - HP:t0]
        xs_np[HP:] = xbf[b, t0:t0 + HALF]
        in_maps.append({
            "xs": xs_np, "bbt": bbt_np, "pc": pc_np, "ct": ct_np,
        })

    _CACHE["last_in_maps"] = in_maps
    if TRACE:
        res = run_bass_kernel_spmd(nc, in_maps, list(range(N_CORES)), trace=True)
        LAST_RESULTS = res
        results = res.results
    else:
        results = _run_spmd_cached(nc, in_maps)

    y = np.empty((BATCH, SEQ, D_MODEL), np.float32)
    for core in range(N_CORES):
        b, half = divmod(core, 2)
        y[b, half * HALF:(half + 1) * HALF, :] = \
            results[core]["ys"].astype(np.float32)
    return y

